# revision 3
# baseline (speedup 1.0000x reference)
"""Trainium2 Bass kernel for nn_Attention_884763263569.

Per-sample compute: k/v projections per view t, q over the concat, 3-way
softmax attention, small FC head.  Pure data-parallel over 8 NeuronCores.

Design (v2, mixed-precision stream):
 - feature-major stage 1: weights stationary (bf16), x streams as the
   moving operand (N=512).  Wfc folded into Wv on host (f = (Wfc@Wv) x);
   stage-1 banks carry [k 0:32 | qp 32:64 | f 64:74].
 - x is split along the 1536-dim contraction into 12 chunks of 128:
   8 chunks ship as bf16, 4 chunks ({1,3,7,11}) as fp8-e3m4.  1.67 B/elem
   = 20.3 MiB/core vs 24.4 bf16-only; the x stream is the HBM roofline
   (~374 GB/s over both HWDGE rings), so bytes ~= time.  Measured rel
   err 1.65e-2 (gate 2e-2); weights stay bf16 (mixed-dtype matmul).
 - intermediates (ysb, ct) in fp16, not bf16: 8x less rounding noise,
   same speed, and the hot epilogue reduces run at 2x on DVE.
 - per 512-sample slab: 12 matmuls accumulate three per-view PSUM banks;
   banks evacuate to SBUF fp16 (ACT 1 + DVE 2), then 12 fp16 PE
   transposes land sample-major in a per-slab PSUM ct buffer
   ([128, 4, 256] fp16 = exactly one bank, double-buffered).
 - softmax epilogue at 1-slab granularity straight out of PSUM; shorter
   tail than the old 2-slab epilogue.
 - PE stream software-pipelined: transposes of slab N ride behind the
   projections of slab N+1.
 - every slab is chunk-split half per HWDGE ring (2 issues per ring per
   slab, 5120 B/partition each) so slab arrivals stay uniform.
 - obuf flushes early for slabs 0..13 on the otherwise-idle SWDGE ring;
   only the last 2 slabs' output rides the final sync DMA -> short tail.
 - dummy-matmul warmup + fill fillers keep the PE HAM clock gate open.
 - host packs x as [slab, 128, chunk, sample] (bf16 and e3m4 tensors)
   and inverts the kernel's sample permutation on the way out (free).
"""

import os
import sys
from contextlib import ExitStack

import numpy as np

sys.path.insert(0, "/opt/trn_rl_repo")

import ml_dtypes

import concourse.bass as bass
import concourse.tile as tile
from concourse import mybir
from concourse.bass_utils import run_bass_kernel_spmd
from concourse.masks import make_identity

# bass_utils imports antenv.axon_hooks unguarded when BASS_TRACE is set; some
# images ship an antenv without that module — stub it so tracing degrades
# gracefully instead of crashing.
try:
    import antenv.axon_hooks  # noqa: F401
except ImportError:
    import types

    import antenv

    _hooks = types.ModuleType("antenv.axon_hooks")
    _hooks._h = None
    _hooks.set_axon_ntff_profile_hook = lambda h: setattr(_hooks, "_h", h)
    _hooks.get_axon_ntff_profile_hook = lambda: _hooks._h
    sys.modules["antenv.axon_hooks"] = _hooks
    antenv.axon_hooks = _hooks


def _register_ctypes_ntff_hook():
    """If no NTFF profile hook is registered, drive profiling via direct
    ctypes calls into libaxon_pjrt.so (slim equivalent of axon.trn's hook;
    same C ABI the boot script uses)."""
    import contextlib
    import ctypes

    from antenv.axon_hooks import (
        get_axon_ntff_profile_hook,
        set_axon_ntff_profile_hook,
    )

    if get_axon_ntff_profile_hook() is not None:
        return
    so_path = os.environ.get("AXON_PJRT_SO", "/opt/axon/libaxon_pjrt.so")
    if not os.path.exists(so_path):
        return
    try:
        lib = ctypes.CDLL(so_path)
    except OSError:
        return
    if not hasattr(lib, "axon_start_nrt_profile"):
        return
    lib.axon_start_nrt_profile.argtypes = [
        ctypes.POINTER(ctypes.c_int64),
        ctypes.c_size_t,
    ]
    lib.axon_start_nrt_profile.restype = ctypes.c_int64
    lib.axon_stop_nrt_profile.argtypes = [ctypes.c_char_p]
    lib.axon_stop_nrt_profile.restype = ctypes.c_int64

    @contextlib.contextmanager
    def _hook(output_dir, device_ids):
        import jax

        jax.devices()
        if device_ids:
            ids = (ctypes.c_int64 * len(device_ids))(*device_ids)
            rc = lib.axon_start_nrt_profile(ids, len(device_ids))
        else:
            rc = lib.axon_start_nrt_profile(None, 0)
        if rc != 0:
            raise RuntimeError(f"axon_start_nrt_profile rc={rc}")
        try:
            yield
        finally:
            n = lib.axon_stop_nrt_profile(str(output_dir).encode())
            print(f"ntff profile: {n} file(s) written to {output_dir}", file=sys.stderr)

    set_axon_ntff_profile_hook(_hook)


try:
    _register_ctypes_ntff_hook()
except Exception:
    pass

BF16 = ml_dtypes.bfloat16
E3M4 = ml_dtypes.float8_e3m4

NCORES = 8
T, D, P, C = 3, 512, 32, 10
DF = T * D            # 1536
KC = DF // 128        # 12 d-chunks
SLAB = 512            # samples per slab (one matmul moving width)
NW = 74               # useful stationary cols: 32 k + 32 qp + 10 f
WARMUP_MM = 32        # dummy matmuls to open the HAM clock gate

CS8 = (1, 3, 7, 11)                      # chunks shipped as fp8-e3m4
CS16 = tuple(c for c in range(KC) if c not in CS8)
IN16 = {c: j for j, c in enumerate(CS16)}
IN8 = {c: j for j, c in enumerate(CS8)}
N16 = len(CS16)
N8 = len(CS8)


def _ins_dim(ap_obj, pos, size, stride=0):
    """Return a new AP with a [stride, size] dim inserted at position pos."""
    new_ap = [list(d) for d in ap_obj.ap]
    new_ap.insert(pos, [stride, size])
    return bass.AP(tensor=ap_obj.tensor, offset=ap_obj.offset, ap=new_ap)


def _remake_ap(ap_obj, dims):
    """Replace the free dims of an AP (keep partition dim)."""
    new_ap = [list(ap_obj.ap[0])] + [list(d) for d in dims]
    return bass.AP(tensor=ap_obj.tensor, offset=ap_obj.offset, ap=new_ap)


def build_nc(nb):
    assert nb % (2 * SLAB) == 0
    nslabs = nb // SLAB

    nc = bass.Bass(target_bir_lowering=False)
    xt16 = nc.declare_dram_parameter(
        "xt16", [nslabs, 128, N16, SLAB], mybir.dt.bfloat16, isOutput=False
    )
    xt8 = nc.declare_dram_parameter(
        "xt8", [nslabs, 128, N8, SLAB], mybir.dt.float8e3, isOutput=False
    )
    wc = nc.declare_dram_parameter("wc", [128, KC, 128], mybir.dt.bfloat16, isOutput=False)
    bfcr = nc.declare_dram_parameter("bfcr", [128, C], mybir.dt.float32, isOutput=False)
    out = nc.declare_dram_parameter(
        "out", [128, nslabs * 4, C], mybir.dt.float32, isOutput=True
    )

    f32 = mybir.dt.float32
    f16 = mybir.dt.float16
    mult = mybir.AluOpType.mult
    add = mybir.AluOpType.add

    with ExitStack() as ctx:
        tc = ctx.enter_context(tile.TileContext(nc))
        wpool = ctx.enter_context(tc.tile_pool(name="wpool", bufs=1))
        xpool16 = ctx.enter_context(tc.tile_pool(name="xpool16", bufs=8))
        xpool8 = ctx.enter_context(tc.tile_pool(name="xpool8", bufs=8))
        ypsum = ctx.enter_context(tc.tile_pool(name="ypsum", bufs=4, space="PSUM"))
        cpsum = ctx.enter_context(tc.tile_pool(name="cpsum", bufs=1, space="PSUM"))
        ypool = ctx.enter_context(tc.tile_pool(name="ypool", bufs=3))
        spool = ctx.enter_context(tc.tile_pool(name="spool", bufs=2))

        xt16_ap = xt16.ap()
        xt8_ap = xt8.ap()
        out_ap = out.ap()

        xtiles = {}

        def prefetch(pf):
            # every slab rides BOTH HWDGE rings (5120 B/partition each, 2
            # issues per ring) so slab arrivals stay uniform.
            if pf < nslabs and pf not in xtiles:
                xs16 = xpool16.tile([128, N16, SLAB], mybir.dt.bfloat16, name="xs16")
                xs8 = xpool8.tile([128, N8, SLAB], mybir.dt.float8e3, name="xs8")
                nc.sync.dma_start(out=xs16[:, 0:4, :], in_=xt16_ap[pf, :, 0:4])
                nc.sync.dma_start(out=xs8[:, 0:2, :], in_=xt8_ap[pf, :, 0:2])
                nc.scalar.dma_start(out=xs16[:, 4:N16, :], in_=xt16_ap[pf, :, 4:N16])
                nc.scalar.dma_start(out=xs8[:, 2:N8, :], in_=xt8_ap[pf, :, 2:N8])
                xtiles[pf] = (xs16, xs8)

        # x slab DMAs issue FIRST so the stream starts as early as possible
        prefetch(0)
        prefetch(1)

        # --- weights split across both HWDGE rings ---
        wc_sb = wpool.tile([128, KC, 128], mybir.dt.bfloat16)
        nc.sync.dma_start(out=wc_sb[:, 0:6, :], in_=wc.ap()[:, 0:6])
        nc.scalar.dma_start(out=wc_sb[:, 6:12, :], in_=wc.ap()[:, 6:12])
        bfc_sb = wpool.tile([128, C], f32)
        nc.gpsimd.dma_start(out=bfc_sb[:], in_=bfcr.ap())

        prefetch(2)
        prefetch(3)
        prefetch(4)
        prefetch(5)

        ident = wpool.tile([128, 128], f16)
        make_identity(nc, ident[:])
        obuf = wpool.tile([128, nslabs * 4, C], f32)

        # per-slab ct buffer: [128, 4 blocks, 256] fp16 = exactly one PSUM
        # bank; per-block layout t*NW+col.  Double-buffered across slabs.
        ct_bufs = [
            cpsum.tile([128, 4, 256], f16, name="ct_a"),
            cpsum.tile([128, 4, 256], f16, name="ct_b"),
        ]

        # --- PE warmup: regular matmuls open the HAM clock gate while the
        # first x slabs stream in
        warm_ps = ypsum.tile([128, SLAB], f32, name="y_ps")
        for i in range(WARMUP_MM):
            nc.tensor.matmul(
                warm_ps[:, 0:128],
                ident[:],
                ident[:],
                start=True,
                stop=True,
                skip_group_check=True,
            )

        def chunk_ap(sl, c):
            """moving-operand AP for chunk c of slab sl."""
            xs16, xs8 = xtiles[sl]
            if c in IN8:
                return xs8[:, IN8[c], :]
            return xs16[:, IN16[c], :]

        ysbs = {}

        def emit_proj(sl):
            prefetch(sl + 6)
            y_t = []
            for t in range(T):
                y_ps = ypsum.tile([128, SLAB], f32, name="y_ps")
                y_t.append(y_ps)
                for i in range(4):
                    c = 4 * t + i
                    nc.tensor.matmul(
                        y_ps[:],
                        wc_sb[:, c, :],
                        chunk_ap(sl, c),
                        start=(i == 0),
                        stop=(i == 3),
                    )
            # evacuate banks to SBUF fp16 (ACT 1, DVE 2 — ACT also carries
            # 2 DMA issues per slab)
            ysb = ypool.tile([128, T, SLAB], f16, name="ysb")
            nc.scalar.copy(out=ysb[:, 0, :], in_=y_t[0][:])
            nc.vector.tensor_copy(out=ysb[:, 1, :], in_=y_t[1][:])
            nc.vector.tensor_copy(out=ysb[:, 2, :], in_=y_t[2][:])
            ysbs[sl] = ysb

        def emit_transposes(sl):
            ct_ps = ct_bufs[sl % 2]
            ysb = ysbs.pop(sl)
            for b in range(4):
                for t in range(T):
                    nc.tensor.transpose(
                        ct_ps[:, b, t * NW : t * NW + NW],
                        ysb[0:NW, t, b * 128 : (b + 1) * 128],
                        ident[0:NW, 0:NW],
                    )

        def emit_epilogue(sl):
            nb8 = 4
            ct_ps = ct_bufs[sl % 2]

            def ctv(c0, c1):
                return _remake_ap(
                    ct_ps[:, 0, c0:c1], [[256, nb8], [NW, 3], [1, c1 - c0]]
                )

            # q[b,p] = sum_t qp[b,t,p] — one reduce with t innermost
            q = spool.tile([128, nb8, P], f16, name="q")
            qp_x = _remake_ap(
                ct_ps[:, 0, 32:64], [[256, nb8], [1, P], [NW, T]]
            )
            with nc.allow_low_precision(reason="fp16 epilogue, |q|<64"):
                nc.vector.tensor_reduce(
                    out=q[:], in_=qp_x, axis=mybir.AxisListType.X, op=add
                )

                # m[b,t,p] = q[b,p] * k[b,t,p]
                m = spool.tile([128, nb8, T, P], f16, name="m")
                q_b = _ins_dim(q[:], 2, T, 0)
                nc.vector.tensor_tensor(out=m[:], in0=q_b, in1=ctv(0, 32), op=mult)
                logits = spool.tile([128, nb8, T], f16, name="l")
                nc.vector.tensor_reduce(
                    out=logits[:], in_=m[:], axis=mybir.AxisListType.X, op=add
                )

            # e = exp(logits) (logits bounded ~±35, no max-subtraction needed)
            e = spool.tile([128, nb8, T], f32, name="e")
            nc.scalar.activation(
                out=e[:], in_=logits[:], func=mybir.ActivationFunctionType.Exp
            )
            z = spool.tile([128, nb8, 1], f32, name="z")
            nc.vector.tensor_reduce(out=z[:], in_=e[:], axis=mybir.AxisListType.X, op=add)
            r = spool.tile([128, nb8, 1], f32, name="r")
            nc.vector.reciprocal(out=r[:], in_=z[:])

            # s[b,f,t] = e[b,t] * fmat[b,t,f]  (written t-innermost)
            s = spool.tile([128, nb8, C, T], f32, name="s")
            e_b = _ins_dim(e[:], 3, C, 0)
            s_out = _remake_ap(s[:], [[C * T, nb8], [1, T], [T, C]])
            nc.vector.tensor_tensor(out=s_out, in0=e_b, in1=ctv(64, 74), op=mult)
            u = spool.tile([128, nb8, C], f32, name="u")
            nc.vector.tensor_reduce(out=u[:], in_=s[:], axis=mybir.AxisListType.X, op=add)

            # out = u * r + bfc
            un = spool.tile([128, nb8, C], f32, name="un")
            r_b = _ins_dim(r[:, :, 0], 2, C, 0)
            nc.vector.tensor_tensor(out=un[:], in0=u[:], in1=r_b, op=mult)
            bfc_b = _ins_dim(bfc_sb[:], 1, nb8, 0)
            nc.vector.tensor_tensor(
                out=obuf[:, sl * 4 : sl * 4 + nb8, :],
                in0=un[:],
                in1=bfc_b,
                op=add,
            )

        # software-pipelined emission: transposes for slab N ride behind
        # the projections of slab N+1 so the strict PE FIFO never waits on
        # the PSUM->SBUF copies.
        def fill_filler(n):
            for _ in range(n):
                nc.tensor.matmul(
                    warm_ps[:, 0:128],
                    ident[:],
                    ident[:],
                    start=True,
                    stop=True,
                    skip_group_check=True,
                )

        nf = (nslabs - 2) * 4  # output blocks flushed early via SWDGE
        emit_proj(0)
        for sl in range(1, nslabs):
            emit_proj(sl)
            emit_transposes(sl - 1)
            emit_epilogue(sl - 1)
            if sl <= 3:
                fill_filler(16)
            if sl == nslabs - 1:
                # early flush of everything but the last 2 slabs on the
                # otherwise-idle SWDGE ring
                nc.gpsimd.dma_start(out=out_ap[:, 0:nf], in_=obuf[:, 0:nf])

        emit_transposes(nslabs - 1)
        emit_epilogue(nslabs - 1)
        nc.sync.dma_start(out=out_ap[:, nf:], in_=obuf[:, nf:])

    nc.finalize()
    _split_excess_waits(nc)
    return nc


def _split_excess_waits(nc):
    """walrus rejects >1 sync wait on compute instruction structs; hoist the
    extras onto same-engine NoOps inserted just before the offender."""
    exempt = (mybir.InstEventSemaphore,)
    for func in nc.m.functions:
        for blk in func.blocks:
            insts = list(blk.instructions)
            out_list = []
            changed = False
            for inst in insts:
                si = getattr(inst, "sync_info", None)
                ow = list(si.on_wait) if (si is not None and si.on_wait) else []
                if len(ow) > 1 and not isinstance(inst, exempt):
                    for w in ow[:-1]:
                        nop = mybir.InstNoOp(
                            name=nc.get_next_instruction_name(),
                            engine=inst.engine,
                            sync_info=mybir.SyncInfo(on_wait=[w], on_update=[]),
                            bass_nofuse=True,
                        )
                        out_list.append(nop)
                    si.on_wait = [ow[-1]]
                    changed = True
                out_list.append(inst)
            if changed:
                blk.instructions = out_list


_NC_CACHE = {}


def _get_nc(nb):
    if nb not in _NC_CACHE:
        _NC_CACHE[nb] = build_nc(nb)
    return _NC_CACHE[nb]


def _prep_weights(Wk, Wv, Wq, Wfc, bfc):
    Wvf = (Wfc.astype(np.float64) @ Wv.astype(np.float64)).astype(np.float32)  # [10,512]
    WkT = Wk.T.astype(np.float32)    # [512, 32]
    WqT = Wq.T.astype(np.float32)    # [1536, 32]
    WvfT = Wvf.T                     # [512, 10]
    wc = np.zeros((KC, 128, 128), np.float32)
    for c in range(KC):
        t, dsub = divmod(c, 4)
        d512 = slice(dsub * 128, (dsub + 1) * 128)
        rows = slice(c * 128, (c + 1) * 128)
        wc[c, :, 0:32] = WkT[d512]
        wc[c, :, 32:64] = WqT[rows]
        wc[c, :, 64:74] = WvfT[d512]
    wc = np.ascontiguousarray(wc.transpose(1, 0, 2)).astype(BF16)  # [128, KC, 128]
    bfcr = np.ascontiguousarray(
        np.broadcast_to(bfc.reshape(1, C).astype(np.float32), (128, C))
    )
    return wc, bfcr


def _pack_x(xr_core, nb):
    # arr[c, p, h, s] = x_cat[h*SLAB + s, 128c + p]
    arr = xr_core.T.reshape(KC, 128, nb // SLAB, SLAB)
    xt16 = np.ascontiguousarray(
        arr[list(CS16)].transpose(2, 1, 0, 3)).astype(BF16)
    xt8 = np.ascontiguousarray(
        arr[list(CS8)].transpose(2, 1, 0, 3)).astype(E3M4)
    return xt16, xt8


def _unpack_out(arr, nb):
    # arr [128, nslabs*4, C]; sample s = h*SLAB + b*128 + p -> arr[p, h*4+b]
    nslabs = nb // SLAB
    return (
        arr.reshape(128, nslabs, 4, C).transpose(1, 2, 0, 3).reshape(nb, C)
    )


LAST_RESULT = None


def kernel(x, Wk, Wv, Wq, Wfc, bfc):
    global LAST_RESULT
    x = np.asarray(x, dtype=np.float32)
    Wk = np.asarray(Wk, dtype=np.float32)
    Wv = np.asarray(Wv, dtype=np.float32)
    Wq = np.asarray(Wq, dtype=np.float32)
    Wfc = np.asarray(Wfc, dtype=np.float32)
    bfc = np.asarray(bfc, dtype=np.float32)

    B = x.shape[0]
    assert B % NCORES == 0
    nb = B // NCORES
    nc = _get_nc(nb)
    wc, bfcr = _prep_weights(Wk, Wv, Wq, Wfc, bfc)

    xr = x.reshape(NCORES, nb, DF)
    in_maps = []
    for i in range(NCORES):
        xt16, xt8 = _pack_x(xr[i], nb)
        in_maps.append({"xt16": xt16, "xt8": xt8, "wc": wc, "bfcr": bfcr})

    LAST_RESULT = run_bass_kernel_spmd(nc, in_maps, core_ids=list(range(NCORES)))
    res = LAST_RESULT.results
    out = np.concatenate(
        [_unpack_out(res[i]["out"], nb) for i in range(NCORES)], axis=0
    )
    return out.astype(np.float32)


# revision 4
# speedup vs baseline: 1.2792x; 1.2792x over previous
"""Trainium2 Bass kernel for nn_Attention_884763263569.

Per-sample compute: k/v projections per view t, q over the concat, 3-way
softmax attention, small FC head.  Pure data-parallel over 8 NeuronCores.

Design (v2, mixed-precision stream):
 - feature-major stage 1: weights stationary (bf16), x streams as the
   moving operand (N=512).  Wfc folded into Wv on host (f = (Wfc@Wv) x);
   stage-1 banks carry [k 0:32 | qp 32:64 | f 64:74].
 - x is split along the 1536-dim contraction into 12 chunks of 128:
   8 chunks ship as bf16, 4 chunks ({1,3,7,11}) as fp8-e3m4.  1.67 B/elem
   = 20.3 MiB/core vs 24.4 bf16-only; the x stream is the HBM roofline
   (~374 GB/s over both HWDGE rings), so bytes ~= time.  Measured rel
   err 1.65e-2 (gate 2e-2); weights stay bf16 (mixed-dtype matmul).
 - intermediates (ysb, ct) in fp16, not bf16: 8x less rounding noise,
   same speed, and the hot epilogue reduces run at 2x on DVE.
 - per 512-sample slab: 12 matmuls accumulate three per-view PSUM banks;
   banks evacuate to SBUF fp16 (ACT 1 + DVE 2), then 12 fp16 PE
   transposes land sample-major in a per-slab PSUM ct buffer
   ([128, 4, 256] fp16 = exactly one bank, double-buffered).
 - softmax epilogue at 1-slab granularity straight out of PSUM; shorter
   tail than the old 2-slab epilogue.
 - PE stream software-pipelined: transposes of slab N ride behind the
   projections of slab N+1.
 - every slab is chunk-split half per HWDGE ring (2 issues per ring per
   slab, 5120 B/partition each) so slab arrivals stay uniform.
 - obuf flushes early for slabs 0..13 on the otherwise-idle SWDGE ring;
   only the last 2 slabs' output rides the final sync DMA -> short tail.
 - dummy-matmul warmup + fill fillers keep the PE HAM clock gate open.
 - host packs x as [slab, 128, chunk, sample] (bf16 and e3m4 tensors)
   and inverts the kernel's sample permutation on the way out (free).
"""

import os
import sys
from contextlib import ExitStack

import numpy as np

sys.path.insert(0, "/opt/trn_rl_repo")

import ml_dtypes

import concourse.bass as bass
import concourse.tile as tile
from concourse import mybir
from concourse.bass_utils import run_bass_kernel_spmd
from concourse.masks import make_identity

# bass_utils imports antenv.axon_hooks unguarded when BASS_TRACE is set; some
# images ship an antenv without that module — stub it so tracing degrades
# gracefully instead of crashing.
try:
    import antenv.axon_hooks  # noqa: F401
except ImportError:
    import types

    import antenv

    _hooks = types.ModuleType("antenv.axon_hooks")
    _hooks._h = None
    _hooks.set_axon_ntff_profile_hook = lambda h: setattr(_hooks, "_h", h)
    _hooks.get_axon_ntff_profile_hook = lambda: _hooks._h
    sys.modules["antenv.axon_hooks"] = _hooks
    antenv.axon_hooks = _hooks


def _register_ctypes_ntff_hook():
    """If no NTFF profile hook is registered, drive profiling via direct
    ctypes calls into libaxon_pjrt.so (slim equivalent of axon.trn's hook;
    same C ABI the boot script uses)."""
    import contextlib
    import ctypes

    from antenv.axon_hooks import (
        get_axon_ntff_profile_hook,
        set_axon_ntff_profile_hook,
    )

    if get_axon_ntff_profile_hook() is not None:
        return
    so_path = os.environ.get("AXON_PJRT_SO", "/opt/axon/libaxon_pjrt.so")
    if not os.path.exists(so_path):
        return
    try:
        lib = ctypes.CDLL(so_path)
    except OSError:
        return
    if not hasattr(lib, "axon_start_nrt_profile"):
        return
    lib.axon_start_nrt_profile.argtypes = [
        ctypes.POINTER(ctypes.c_int64),
        ctypes.c_size_t,
    ]
    lib.axon_start_nrt_profile.restype = ctypes.c_int64
    lib.axon_stop_nrt_profile.argtypes = [ctypes.c_char_p]
    lib.axon_stop_nrt_profile.restype = ctypes.c_int64

    @contextlib.contextmanager
    def _hook(output_dir, device_ids):
        import jax

        jax.devices()
        if device_ids:
            ids = (ctypes.c_int64 * len(device_ids))(*device_ids)
            rc = lib.axon_start_nrt_profile(ids, len(device_ids))
        else:
            rc = lib.axon_start_nrt_profile(None, 0)
        if rc != 0:
            raise RuntimeError(f"axon_start_nrt_profile rc={rc}")
        try:
            yield
        finally:
            n = lib.axon_stop_nrt_profile(str(output_dir).encode())
            print(f"ntff profile: {n} file(s) written to {output_dir}", file=sys.stderr)

    set_axon_ntff_profile_hook(_hook)


try:
    _register_ctypes_ntff_hook()
except Exception:
    pass

BF16 = ml_dtypes.bfloat16
E3M4 = ml_dtypes.float8_e3m4

NCORES = 8
T, D, P, C = 3, 512, 32, 10
DF = T * D            # 1536
KC = DF // 128        # 12 d-chunks
SLAB = 512            # samples per slab (one matmul moving width)
NW = 74               # useful stationary cols: 32 k + 32 qp + 10 f
WARMUP_MM = 32        # dummy matmuls to open the HAM clock gate

CS8 = (1, 3, 7, 11)                      # chunks shipped as fp8-e3m4
CS16 = tuple(c for c in range(KC) if c not in CS8)
IN16 = {c: j for j, c in enumerate(CS16)}
IN8 = {c: j for j, c in enumerate(CS8)}
N16 = len(CS16)
N8 = len(CS8)


def _ins_dim(ap_obj, pos, size, stride=0):
    """Return a new AP with a [stride, size] dim inserted at position pos."""
    new_ap = [list(d) for d in ap_obj.ap]
    new_ap.insert(pos, [stride, size])
    return bass.AP(tensor=ap_obj.tensor, offset=ap_obj.offset, ap=new_ap)


def _remake_ap(ap_obj, dims):
    """Replace the free dims of an AP (keep partition dim)."""
    new_ap = [list(ap_obj.ap[0])] + [list(d) for d in dims]
    return bass.AP(tensor=ap_obj.tensor, offset=ap_obj.offset, ap=new_ap)


def build_nc(nb):
    assert nb % (2 * SLAB) == 0
    nslabs = nb // SLAB

    nc = bass.Bass(target_bir_lowering=False)
    xt16 = nc.declare_dram_parameter(
        "xt16", [nslabs, 128, N16, SLAB], mybir.dt.bfloat16, isOutput=False
    )
    xt8 = nc.declare_dram_parameter(
        "xt8", [nslabs, 128, N8, SLAB], mybir.dt.float8e3, isOutput=False
    )
    wc = nc.declare_dram_parameter("wc", [128, KC, 128], mybir.dt.bfloat16, isOutput=False)
    bfcr = nc.declare_dram_parameter("bfcr", [128, C], mybir.dt.float32, isOutput=False)
    out = nc.declare_dram_parameter(
        "out", [128, nslabs * 4, C], mybir.dt.float32, isOutput=True
    )

    f32 = mybir.dt.float32
    f16 = mybir.dt.float16
    mult = mybir.AluOpType.mult
    add = mybir.AluOpType.add

    with ExitStack() as ctx:
        tc = ctx.enter_context(tile.TileContext(nc))
        wpool = ctx.enter_context(tc.tile_pool(name="wpool", bufs=1))
        xpool16 = ctx.enter_context(tc.tile_pool(name="xpool16", bufs=8))
        xpool8 = ctx.enter_context(tc.tile_pool(name="xpool8", bufs=8))
        ypsum = ctx.enter_context(tc.tile_pool(name="ypsum", bufs=4, space="PSUM"))
        cpsum = ctx.enter_context(tc.tile_pool(name="cpsum", bufs=1, space="PSUM"))
        ypool = ctx.enter_context(tc.tile_pool(name="ypool", bufs=3))
        spool = ctx.enter_context(tc.tile_pool(name="spool", bufs=2))

        xt16_ap = xt16.ap()
        xt8_ap = xt8.ap()
        out_ap = out.ap()

        xtiles = {}

        def prefetch(pf):
            # one LARGE dma_start per tensor, alternating rings per slab:
            # ring A gets slab N's bf16 + slab N+1's fp8, ring B the
            # converse — big transfers keep per-queue BW at ~187 GB/s and
            # bytes stay balanced across rings at 2-slab granularity.
            if pf < nslabs and pf not in xtiles:
                xs16 = xpool16.tile([128, N16, SLAB], mybir.dt.bfloat16, name="xs16")
                xs8 = xpool8.tile([128, N8, SLAB], mybir.dt.float8e3, name="xs8")
                ring16, ring8 = (
                    (nc.sync, nc.scalar) if pf % 2 == 0 else (nc.scalar, nc.sync)
                )
                ring16.dma_start(out=xs16[:], in_=xt16_ap[pf])
                ring8.dma_start(out=xs8[:], in_=xt8_ap[pf])
                xtiles[pf] = (xs16, xs8)

        # x slab DMAs issue FIRST so the stream starts as early as possible
        prefetch(0)
        prefetch(1)

        # --- weights split across both HWDGE rings ---
        wc_sb = wpool.tile([128, KC, 128], mybir.dt.bfloat16)
        nc.sync.dma_start(out=wc_sb[:, 0:6, :], in_=wc.ap()[:, 0:6])
        nc.scalar.dma_start(out=wc_sb[:, 6:12, :], in_=wc.ap()[:, 6:12])
        bfc_sb = wpool.tile([128, C], f32)
        nc.gpsimd.dma_start(out=bfc_sb[:], in_=bfcr.ap())

        prefetch(2)
        prefetch(3)
        prefetch(4)
        prefetch(5)

        ident = wpool.tile([128, 128], f16)
        make_identity(nc, ident[:])
        obuf = wpool.tile([128, nslabs * 4, C], f32)

        # per-slab ct buffer: [128, 4 blocks, 256] fp16 = exactly one PSUM
        # bank; per-block layout t*NW+col.  Double-buffered across slabs.
        ct_bufs = [
            cpsum.tile([128, 4, 256], f16, name="ct_a"),
            cpsum.tile([128, 4, 256], f16, name="ct_b"),
        ]

        # --- PE warmup: regular matmuls open the HAM clock gate while the
        # first x slabs stream in
        warm_ps = ypsum.tile([128, SLAB], f32, name="y_ps")
        for i in range(WARMUP_MM):
            nc.tensor.matmul(
                warm_ps[:, 0:128],
                ident[:],
                ident[:],
                start=True,
                stop=True,
                skip_group_check=True,
            )

        def chunk_ap(sl, c):
            """moving-operand AP for chunk c of slab sl."""
            xs16, xs8 = xtiles[sl]
            if c in IN8:
                return xs8[:, IN8[c], :]
            return xs16[:, IN16[c], :]

        ysbs = {}

        def emit_proj(sl):
            prefetch(sl + 6)
            y_t = []
            for t in range(T):
                y_ps = ypsum.tile([128, SLAB], f32, name="y_ps")
                y_t.append(y_ps)
                for i in range(4):
                    c = 4 * t + i
                    nc.tensor.matmul(
                        y_ps[:],
                        wc_sb[:, c, :],
                        chunk_ap(sl, c),
                        start=(i == 0),
                        stop=(i == 3),
                    )
            # evacuate banks to SBUF fp16 (ACT 1, DVE 2 — ACT also carries
            # 2 DMA issues per slab)
            ysb = ypool.tile([128, T, SLAB], f16, name="ysb")
            nc.scalar.copy(out=ysb[:, 0, :], in_=y_t[0][:])
            nc.vector.tensor_copy(out=ysb[:, 1, :], in_=y_t[1][:])
            nc.vector.tensor_copy(out=ysb[:, 2, :], in_=y_t[2][:])
            ysbs[sl] = ysb

        def emit_transposes(sl):
            ct_ps = ct_bufs[sl % 2]
            ysb = ysbs.pop(sl)
            for b in range(4):
                for t in range(T):
                    nc.tensor.transpose(
                        ct_ps[:, b, t * NW : t * NW + NW],
                        ysb[0:NW, t, b * 128 : (b + 1) * 128],
                        ident[0:NW, 0:NW],
                    )

        def emit_epilogue(sl):
            nb8 = 4
            ct_ps = ct_bufs[sl % 2]

            def ctv(c0, c1):
                return _remake_ap(
                    ct_ps[:, 0, c0:c1], [[256, nb8], [NW, 3], [1, c1 - c0]]
                )

            # q[b,p] = sum_t qp[b,t,p] — one reduce with t innermost
            q = spool.tile([128, nb8, P], f16, name="q")
            qp_x = _remake_ap(
                ct_ps[:, 0, 32:64], [[256, nb8], [1, P], [NW, T]]
            )
            with nc.allow_low_precision(reason="fp16 epilogue, |q|<64"):
                nc.vector.tensor_reduce(
                    out=q[:], in_=qp_x, axis=mybir.AxisListType.X, op=add
                )

                # m[b,t,p] = q[b,p] * k[b,t,p]
                m = spool.tile([128, nb8, T, P], f16, name="m")
                q_b = _ins_dim(q[:], 2, T, 0)
                nc.vector.tensor_tensor(out=m[:], in0=q_b, in1=ctv(0, 32), op=mult)
                logits = spool.tile([128, nb8, T], f16, name="l")
                nc.vector.tensor_reduce(
                    out=logits[:], in_=m[:], axis=mybir.AxisListType.X, op=add
                )

            # e = exp(logits) (logits bounded ~±35, no max-subtraction needed)
            e = spool.tile([128, nb8, T], f32, name="e")
            nc.scalar.activation(
                out=e[:], in_=logits[:], func=mybir.ActivationFunctionType.Exp
            )
            z = spool.tile([128, nb8, 1], f32, name="z")
            nc.vector.tensor_reduce(out=z[:], in_=e[:], axis=mybir.AxisListType.X, op=add)
            r = spool.tile([128, nb8, 1], f32, name="r")
            nc.vector.reciprocal(out=r[:], in_=z[:])

            # s[b,f,t] = e[b,t] * fmat[b,t,f]  (written t-innermost)
            s = spool.tile([128, nb8, C, T], f32, name="s")
            e_b = _ins_dim(e[:], 3, C, 0)
            s_out = _remake_ap(s[:], [[C * T, nb8], [1, T], [T, C]])
            nc.vector.tensor_tensor(out=s_out, in0=e_b, in1=ctv(64, 74), op=mult)
            u = spool.tile([128, nb8, C], f32, name="u")
            nc.vector.tensor_reduce(out=u[:], in_=s[:], axis=mybir.AxisListType.X, op=add)

            # out = u * r + bfc
            un = spool.tile([128, nb8, C], f32, name="un")
            r_b = _ins_dim(r[:, :, 0], 2, C, 0)
            nc.vector.tensor_tensor(out=un[:], in0=u[:], in1=r_b, op=mult)
            bfc_b = _ins_dim(bfc_sb[:], 1, nb8, 0)
            nc.vector.tensor_tensor(
                out=obuf[:, sl * 4 : sl * 4 + nb8, :],
                in0=un[:],
                in1=bfc_b,
                op=add,
            )

        # software-pipelined emission: transposes for slab N ride behind
        # the projections of slab N+1 so the strict PE FIFO never waits on
        # the PSUM->SBUF copies.
        def fill_filler(n):
            for _ in range(n):
                nc.tensor.matmul(
                    warm_ps[:, 0:128],
                    ident[:],
                    ident[:],
                    start=True,
                    stop=True,
                    skip_group_check=True,
                )

        nf = (nslabs - 2) * 4  # output blocks flushed early via SWDGE
        emit_proj(0)
        for sl in range(1, nslabs):
            emit_proj(sl)
            emit_transposes(sl - 1)
            emit_epilogue(sl - 1)
            if sl <= 3:
                fill_filler(16)
            if sl == nslabs - 1:
                # early flush of everything but the last 2 slabs on the
                # otherwise-idle SWDGE ring
                nc.gpsimd.dma_start(out=out_ap[:, 0:nf], in_=obuf[:, 0:nf])

        emit_transposes(nslabs - 1)
        emit_epilogue(nslabs - 1)
        nc.sync.dma_start(out=out_ap[:, nf:], in_=obuf[:, nf:])

    nc.finalize()
    _split_excess_waits(nc)
    return nc


def _split_excess_waits(nc):
    """walrus rejects >1 sync wait on compute instruction structs; hoist the
    extras onto same-engine NoOps inserted just before the offender."""
    exempt = (mybir.InstEventSemaphore,)
    for func in nc.m.functions:
        for blk in func.blocks:
            insts = list(blk.instructions)
            out_list = []
            changed = False
            for inst in insts:
                si = getattr(inst, "sync_info", None)
                ow = list(si.on_wait) if (si is not None and si.on_wait) else []
                if len(ow) > 1 and not isinstance(inst, exempt):
                    for w in ow[:-1]:
                        nop = mybir.InstNoOp(
                            name=nc.get_next_instruction_name(),
                            engine=inst.engine,
                            sync_info=mybir.SyncInfo(on_wait=[w], on_update=[]),
                            bass_nofuse=True,
                        )
                        out_list.append(nop)
                    si.on_wait = [ow[-1]]
                    changed = True
                out_list.append(inst)
            if changed:
                blk.instructions = out_list


_NC_CACHE = {}


def _get_nc(nb):
    if nb not in _NC_CACHE:
        _NC_CACHE[nb] = build_nc(nb)
    return _NC_CACHE[nb]


def _prep_weights(Wk, Wv, Wq, Wfc, bfc):
    Wvf = (Wfc.astype(np.float64) @ Wv.astype(np.float64)).astype(np.float32)  # [10,512]
    WkT = Wk.T.astype(np.float32)    # [512, 32]
    WqT = Wq.T.astype(np.float32)    # [1536, 32]
    WvfT = Wvf.T                     # [512, 10]
    wc = np.zeros((KC, 128, 128), np.float32)
    for c in range(KC):
        t, dsub = divmod(c, 4)
        d512 = slice(dsub * 128, (dsub + 1) * 128)
        rows = slice(c * 128, (c + 1) * 128)
        wc[c, :, 0:32] = WkT[d512]
        wc[c, :, 32:64] = WqT[rows]
        wc[c, :, 64:74] = WvfT[d512]
    wc = np.ascontiguousarray(wc.transpose(1, 0, 2)).astype(BF16)  # [128, KC, 128]
    bfcr = np.ascontiguousarray(
        np.broadcast_to(bfc.reshape(1, C).astype(np.float32), (128, C))
    )
    return wc, bfcr


def _pack_x(xr_core, nb):
    # arr[c, p, h, s] = x_cat[h*SLAB + s, 128c + p]
    arr = xr_core.T.reshape(KC, 128, nb // SLAB, SLAB)
    xt16 = np.ascontiguousarray(
        arr[list(CS16)].transpose(2, 1, 0, 3)).astype(BF16)
    xt8 = np.ascontiguousarray(
        arr[list(CS8)].transpose(2, 1, 0, 3)).astype(E3M4)
    return xt16, xt8


def _unpack_out(arr, nb):
    # arr [128, nslabs*4, C]; sample s = h*SLAB + b*128 + p -> arr[p, h*4+b]
    nslabs = nb // SLAB
    return (
        arr.reshape(128, nslabs, 4, C).transpose(1, 2, 0, 3).reshape(nb, C)
    )


LAST_RESULT = None


def kernel(x, Wk, Wv, Wq, Wfc, bfc):
    global LAST_RESULT
    x = np.asarray(x, dtype=np.float32)
    Wk = np.asarray(Wk, dtype=np.float32)
    Wv = np.asarray(Wv, dtype=np.float32)
    Wq = np.asarray(Wq, dtype=np.float32)
    Wfc = np.asarray(Wfc, dtype=np.float32)
    bfc = np.asarray(bfc, dtype=np.float32)

    B = x.shape[0]
    assert B % NCORES == 0
    nb = B // NCORES
    nc = _get_nc(nb)
    wc, bfcr = _prep_weights(Wk, Wv, Wq, Wfc, bfc)

    xr = x.reshape(NCORES, nb, DF)
    in_maps = []
    for i in range(NCORES):
        xt16, xt8 = _pack_x(xr[i], nb)
        in_maps.append({"xt16": xt16, "xt8": xt8, "wc": wc, "bfcr": bfcr})

    LAST_RESULT = run_bass_kernel_spmd(nc, in_maps, core_ids=list(range(NCORES)))
    res = LAST_RESULT.results
    out = np.concatenate(
        [_unpack_out(res[i]["out"], nb) for i in range(NCORES)], axis=0
    )
    return out.astype(np.float32)
